# revision 18
# baseline (speedup 1.0000x reference)
"""Adder2D (L1-distance "convolution") Trainium2 Bass kernel, 8 NeuronCores.

out[n, f, ho, wo] = -sum_d |W[f, d] - X_col[d, (n, ho, wo)]|
with d = (c, dy, dx), C=128, 3x3 kernel, stride 1, pad 1.

Sharding: output-channel tensor parallel. Core i computes filters
[16*i, 16*(i+1)); every core sees the full x. No collectives; the host
concatenates the 8 per-core outputs along the filter axis.

Per-core algorithm (relu identity, exact):
  |x-w| = 2*relu(x-w) - (x-w)
  out[f, l] = -2*sum_d relu(x - w[f,d]) + S_X[l] - S_W[f]
  S_X[l] = sum_d x[d, l],  S_W[f] = sum_d w[f, d]

  - x (f32) is DMA'd contiguously, then zero-padded into
    [128c, 8n*18*18] on-chip.
  - 9 shifted copies materialize im2col patches as bf16 [128c, 2048l];
    the "unfold" is just an access pattern (center patch first: it has
    no dependency on the padding, so the PE pipeline starts early).
  - relu tiles: DVE tensor_scalar(op0=subtract, op1=max, 0.0) with a
    per-partition f32 W scalar (4x bf16 path), ~75% of tiles; ACT
    activation(Relu, bias=-w) for the rest.
  - TensorEngine reduces over partitions with accumulating matmuls into
    one [16, 2048] PSUM tile; stationary = [128,16] column of -2 at
    column f. A custom pass drops LDWEIGHTS whose stationary is
    unchanged (else walrus reloads it for every matmul: +38% PE time).
  - S_X: tree-add of the 9 patches (DVE/GpSimd) + a ones-column matmul;
    S_W: ones-column matmul over W + a 9-fold strided add. Both are
    broadcast back into the PSUM accumulator with K=1 matmuls, so the
    corrections cost ~13 matmuls instead of 72.
"""

import numpy as np

N, C, H, W_ = 8, 128, 16, 16
F, KH, KW = 128, 3, 3
NCORES = 8
FL = F // NCORES          # 16 filters per core
HP, WP = H + 2, W_ + 2    # padded 18x18
L = N * H * W_            # 2048 output columns
DCH = KH * KW             # 9 shift chunks of 128 channels
NT = 512                  # matmul moving free dim (one PSUM bank)
ACT_MOD, ACT_K = 10, 3    # 3 of every 10 relu tiles run on the Scalar engine
WARMUP_MM = 24            # PE warmup matmuls during the DMA/setup phase
J_ORDER = [4, 0, 1, 2, 3, 5, 6, 7, 8]   # center shift first

_CACHE = {}


def _dedup_ldweights(nc):
    """Drop InstLdweights whose stationary operand is identical to the
    previous weight load on the PE stream (the array keeps its weights
    between matmuls; per-matmul reloads of an unchanged stationary are
    pure overhead). Runs after Tile scheduling, before bacc.compile,
    when the ldweights carry no semaphore sync."""
    from concourse import mybir
    removed = 0
    for fn in nc.m.functions:
        for blk in fn.blocks:
            last_key = None
            keep = []
            for inst in blk.instructions:
                if isinstance(inst, mybir.InstLdweights):
                    si = inst.sync_info
                    clean = si is None or (not si.on_wait and not si.on_update)
                    key = "|".join(str(s) for s in (
                        inst.ins[0], inst.perf_mode, inst.is_transpose,
                        inst.tile_position, inst.tile_size))
                    if clean and key == last_key:
                        removed += 1
                        continue
                    last_key = key
                keep.append(inst)
            blk.instructions[:] = keep
    return removed


def _build_nc():
    from concourse import bacc, mybir
    import concourse.tile as tile

    f32 = mybir.dt.float32
    bf16 = mybir.dt.bfloat16
    Alu = mybir.AluOpType
    Act = mybir.ActivationFunctionType

    nc = bacc.Bacc("TRN2", target_bir_lowering=False, debug=False,
                   num_devices=NCORES)
    x_d = nc.dram_tensor("x", [N, C, H, W_], f32, kind="ExternalInput")
    w_d = nc.dram_tensor("w", [FL, C, KH, KW], f32, kind="ExternalInput")
    out_d = nc.dram_tensor("out", [N, FL, H, W_], f32, kind="ExternalOutput")

    with tile.TileContext(nc) as tc:
        with tc.tile_pool(name="setup", bufs=1) as sp, \
             tc.tile_pool(name="diff", bufs=6) as dp, \
             tc.tile_pool(name="psum", bufs=1, space="PSUM") as pp:

            # ---- x: contiguous DMA ----
            x_flat = sp.tile([128, L], f32)
            xsrc = x_d.ap().rearrange("n c h w -> c n (h w)")
            nc.sync.dma_start(
                x_flat[:].rearrange("p (n hw) -> p n hw", n=N), xsrc)

            # ---- W: [c, (f j)] layout keeps the DMA descriptors 36B ----
            w32 = sp.tile([128, FL * DCH], f32)
            wsrc = w_d.ap().rearrange("f c kh kw -> c f (kh kw)")
            nc.sync.dma_start(
                w32[:].rearrange("p (f j) -> p f j", f=FL), wsrc)
            wb = sp.tile([128, FL * DCH], bf16)
            nc.gpsimd.tensor_copy(wb[:], w32[:])
            nc.gpsimd.tensor_copy(w32[:], wb[:])
            w32n = sp.tile([128, FL * DCH], f32)
            nc.gpsimd.tensor_scalar(w32n[:], w32[:], -1.0, None, op0=Alu.mult)
            w32_3 = w32[:].rearrange("p (f j) -> p f j", f=FL)
            w32n_3 = w32n[:].rearrange("p (f j) -> p f j", f=FL)

            # ---- stationary / constant tiles ----
            ind = sp.tile([128, FL * FL], bf16)   # -2 at column f
            nc.gpsimd.memset(ind[:], 0.0)
            ind3 = ind[:].rearrange("p (f m) -> p f m", f=FL)
            for f in range(FL):
                nc.gpsimd.memset(ind3[:, f, f:f + 1], -2.0)
            ones_st = sp.tile([128, FL], bf16)
            nc.gpsimd.memset(ones_st[:], 1.0)
            neg_ones = sp.tile([128, NT], bf16)
            nc.gpsimd.memset(neg_ones[:], -1.0)

            # ---- PE warmup while DMA/pad runs (keeps HAM at 2.4 GHz) ----
            warm = pp.tile([FL, NT], f32, tag="aux")
            for i in range(WARMUP_MM):
                nc.tensor.matmul(warm[:], ones_st[:], neg_ones[:],
                                 start=(i == 0), stop=(i == WARMUP_MM - 1))

            # ---- padded x ----
            x_pad = sp.tile([128, N * HP * WP], f32)
            nc.gpsimd.memset(x_pad[:], 0.0)
            x_pad4 = x_pad[:].rearrange("p (n h w) -> p n h w", n=N, h=HP, w=WP)
            nc.scalar.copy(
                x_pad4[:, :, 1:1 + H, 1:1 + W_],
                x_flat[:].rearrange("p (n h w) -> p n h w", n=N, h=H, w=W_))

            # ---- the 9 shifted patch tiles (bf16), center first ----
            patches = [None] * DCH
            for k, j in enumerate(J_ORDER):
                dy, dx = divmod(j, KW)
                pj = sp.tile([128, L], bf16, tag=f"patch{j}")
                if j == 4:
                    nc.vector.tensor_copy(pj[:], x_flat[:])
                else:
                    pj4 = pj[:].rearrange(
                        "p (n h w) -> p n h w", n=N, h=H, w=W_)
                    src = x_pad4[:, :, dy:dy + H, dx:dx + W_]
                    if k % 2 == 1:
                        nc.vector.tensor_copy(pj4, src)
                    else:
                        nc.scalar.copy(pj4, src)
                patches[j] = pj

            # ---- S_X inputs: tree-add the 9 patches (early, so the
            #      correction matmuls at the end never stall) ----
            s01 = sp.tile([128, L], bf16)
            s23 = sp.tile([128, L], bf16)
            s56 = sp.tile([128, L], bf16)
            s78 = sp.tile([128, L], bf16)
            nc.gpsimd.tensor_tensor(s23[:], patches[2][:], patches[3][:],
                                    op=Alu.add)
            nc.gpsimd.tensor_tensor(s78[:], patches[7][:], patches[8][:],
                                    op=Alu.add)
            nc.vector.tensor_tensor(s01[:], patches[0][:], patches[1][:],
                                    op=Alu.add)
            nc.vector.tensor_tensor(s56[:], patches[5][:], patches[6][:],
                                    op=Alu.add)
            nc.vector.tensor_tensor(s01[:], s01[:], s23[:], op=Alu.add)
            nc.vector.tensor_tensor(s56[:], s56[:], s78[:], op=Alu.add)
            nc.vector.tensor_tensor(s01[:], s01[:], s56[:], op=Alu.add)
            nc.vector.tensor_tensor(s01[:], s01[:], patches[4][:], op=Alu.add)

            psum = pp.tile([FL, L], f32)
            nchunks = L // NT

            # S_X: reduce sum-of-patches over partitions
            sxp = pp.tile([1, L], f32, tag="aux")
            for ncnk in range(nchunks):
                cs = slice(ncnk * NT, (ncnk + 1) * NT)
                nc.tensor.matmul(sxp[:, cs], ones_st[:, 0:1], s01[:, cs],
                                 start=True, stop=True)
            sxb = sp.tile([1, L], bf16)
            nc.scalar.copy(sxb[:], sxp[:])

            # S_W: reduce W over partitions, fold the 9 taps
            swp = pp.tile([1, FL * DCH], f32, tag="aux")
            nc.tensor.matmul(swp[:], ones_st[:, 0:1], wb[:],
                             start=True, stop=True)
            swf = sp.tile([1, FL * DCH], f32)
            nc.scalar.copy(swf[:], swp[:])
            swf3 = swf[:].rearrange("p (f j) -> p f j", f=FL)
            for k in range(1, DCH):
                nc.vector.tensor_tensor(
                    swf3[:, :, 0], swf3[:, :, 0], swf3[:, :, k], op=Alu.add)
            swb = sp.tile([1, FL], bf16)
            nc.vector.tensor_copy(swb[:], swf3[:, :, 0])

            # ---- main loop: relu tiles -> accumulating matmuls ----
            first = [True] * nchunks
            for f in range(FL):
                lhsT = ind3[:, f, :]
                for j in J_ORDER:
                    dt_ = dp.tile([128, L], bf16, tag="diff")
                    if (f * DCH + j) % ACT_MOD < ACT_K:
                        nc.scalar.activation(
                            dt_[:], patches[j][:], Act.Relu,
                            bias=w32n_3[:, f, j:j + 1], scale=1.0)
                    else:
                        nc.vector.tensor_scalar(
                            dt_[:], patches[j][:], w32_3[:, f, j:j + 1], 0.0,
                            op0=Alu.subtract, op1=Alu.max)
                    for ncnk in range(nchunks):
                        cs = slice(ncnk * NT, (ncnk + 1) * NT)
                        nc.tensor.matmul(
                            psum[:, cs], lhsT, dt_[:, cs],
                            start=first[ncnk], stop=False)
                        first[ncnk] = False

            # ---- broadcast corrections into psum with K=1 matmuls,
            #      then stream each finished chunk straight out ----
            osb = sp.tile([FL, L], f32)
            odst = out_d.ap().rearrange("n f h w -> f n (h w)")
            osb3 = osb[:].rearrange("f (n hw) -> f n hw", n=N)
            for ncnk in range(nchunks):
                cs = slice(ncnk * NT, (ncnk + 1) * NT)
                nc.tensor.matmul(                      # += S_X[l] every row
                    psum[:, cs], ones_st[0:1, :], sxb[:, cs],
                    start=False, stop=False)
                nc.tensor.matmul(                      # += -S_W[f] every col
                    psum[:, cs], swb[:], neg_ones[0:1, :],
                    start=False, stop=True)
                nc.scalar.copy(osb[:, cs], psum[:, cs])
            nc.sync.dma_start(odst, osb3)

    _dedup_ldweights(nc)
    nc.compile()
    return nc


def kernel(x, W):
    x = np.ascontiguousarray(np.asarray(x, dtype=np.float32))
    W = np.ascontiguousarray(np.asarray(W, dtype=np.float32))
    assert x.shape == (N, C, H, W_) and W.shape == (F, C, KH, KW)

    if "nc" not in _CACHE:
        _CACHE["nc"] = _build_nc()
    nc = _CACHE["nc"]

    from concourse.bass_utils import run_bass_kernel_spmd

    in_maps = [
        {"x": x, "w": np.ascontiguousarray(W[FL * i:FL * (i + 1)])}
        for i in range(NCORES)
    ]
    trace = bool(_CACHE.get("trace", False))
    res = run_bass_kernel_spmd(nc, in_maps, core_ids=list(range(NCORES)),
                               trace=trace)
    _CACHE["exec_time_ns"] = res.exec_time_ns
    out = np.concatenate([r["out"] for r in res.results], axis=1)
    return out.astype(np.float32)


# revision 23
# speedup vs baseline: 1.0176x; 1.0176x over previous
"""Adder2D (L1-distance "convolution") Trainium2 Bass kernel, 8 NeuronCores.

out[n, f, ho, wo] = -sum_d |W[f, d] - X_col[d, (n, ho, wo)]|
with d = (c, dy, dx), C=128, 3x3 kernel, stride 1, pad 1.

Sharding: output-channel tensor parallel. Core i computes filters
[16*i, 16*(i+1)); every core sees the full x. No collectives; the host
concatenates the 8 per-core outputs along the filter axis.

Per-core algorithm (relu identity, exact):
  |x-w| = 2*relu(x-w) - (x-w)
  out[f, l] = -2*sum_d relu(x - w[f,d]) + S_X[l] - S_W[f]
  S_X[l] = sum_d x[d, l],  S_W[f] = sum_d w[f, d]

  - x (f32) is DMA'd contiguously, then zero-padded into
    [128c, 8n*18*18] on-chip.
  - 9 shifted copies materialize im2col patches as bf16 [128c, 2048l];
    the "unfold" is just an access pattern (center patch first: it has
    no dependency on the padding, so the PE pipeline starts early).
  - relu tiles: DVE tensor_scalar(op0=subtract, op1=max, 0.0) with a
    per-partition f32 W scalar (4x bf16 path), ~75% of tiles; ACT
    activation(Relu, bias=-w) for the rest.
  - TensorEngine reduces over partitions with accumulating matmuls into
    one [16, 2048] PSUM tile; stationary = [128,16] column of -2 at
    column f. A custom pass drops LDWEIGHTS whose stationary is
    unchanged (else walrus reloads it for every matmul: +38% PE time).
  - S_X: tree-add of the 9 patches (DVE/GpSimd) + a ones-column matmul;
    S_W: ones-column matmul over W + a 9-fold strided add. Both are
    broadcast back into the PSUM accumulator with K=1 matmuls, so the
    corrections cost ~13 matmuls instead of 72.
"""

import numpy as np

N, C, H, W_ = 8, 128, 16, 16
F, KH, KW = 128, 3, 3
NCORES = 8
FL = F // NCORES          # 16 filters per core
HP, WP = H + 2, W_ + 2    # padded 18x18
L = N * H * W_            # 2048 output columns
DCH = KH * KW             # 9 shift chunks of 128 channels
NT = 512                  # matmul moving free dim (one PSUM bank)
ACT_MOD, ACT_K = 3, 1     # 1 of every 3 relu tiles runs on the Scalar engine
WARMUP_MM = 16            # PE warmup matmuls during the DMA/setup phase
MERGE_PAIRS = [(0, 1), (2, 3)]          # relu tiles pre-added on DVE
SINGLE_J = [4, 5, 6, 7, 8]              # tiles fed to PE directly
J_ORDER = [4, 0, 1, 2, 3, 5, 6, 7, 8]   # center shift first (patch creation)

_CACHE = {}


def _dedup_ldweights(nc):
    """Drop InstLdweights whose stationary operand is identical to the
    previous weight load on the PE stream (the array keeps its weights
    between matmuls; per-matmul reloads of an unchanged stationary are
    pure overhead). Runs after Tile scheduling, before bacc.compile,
    when the ldweights carry no semaphore sync."""
    from concourse import mybir
    removed = 0
    for fn in nc.m.functions:
        for blk in fn.blocks:
            last_key = None
            keep = []
            for inst in blk.instructions:
                if isinstance(inst, mybir.InstLdweights):
                    si = inst.sync_info
                    clean = si is None or (not si.on_wait and not si.on_update)
                    key = "|".join(str(s) for s in (
                        inst.ins[0], inst.perf_mode, inst.is_transpose,
                        inst.tile_position, inst.tile_size))
                    if clean and key == last_key:
                        removed += 1
                        continue
                    last_key = key
                keep.append(inst)
            blk.instructions[:] = keep
    return removed


def _build_nc():
    from concourse import bacc, mybir
    import concourse.tile as tile

    f32 = mybir.dt.float32
    bf16 = mybir.dt.bfloat16
    Alu = mybir.AluOpType
    Act = mybir.ActivationFunctionType

    nc = bacc.Bacc("TRN2", target_bir_lowering=False, debug=False,
                   num_devices=NCORES)
    x_d = nc.dram_tensor("x", [N, C, H, W_], f32, kind="ExternalInput")
    w_d = nc.dram_tensor("w", [FL, C, KH, KW], f32, kind="ExternalInput")
    out_d = nc.dram_tensor("out", [N, FL, H, W_], f32, kind="ExternalOutput")

    with tile.TileContext(nc) as tc:
        with tc.tile_pool(name="setup", bufs=1) as sp, \
             tc.tile_pool(name="diff", bufs=8) as dp, \
             tc.tile_pool(name="psum", bufs=1, space="PSUM") as pp:

            # ---- PE warmup first: constants on DVE, then matmuls that
            #      keep HAM at 2.4 GHz while the DMAs/setup run ----
            ones_st = sp.tile([128, FL], bf16)
            nc.vector.memset(ones_st[:], 1.0)
            neg_ones = sp.tile([128, NT], bf16)
            nc.vector.memset(neg_ones[:], -1.0)
            warm = pp.tile([FL, NT], f32, tag="aux")
            for i in range(WARMUP_MM):
                nc.tensor.matmul(warm[:], ones_st[:], neg_ones[:],
                                 start=(i == 0), stop=(i == WARMUP_MM - 1))

            # ---- x: contiguous DMA ----
            x_flat = sp.tile([128, L], f32)
            xsrc = x_d.ap().rearrange("n c h w -> c n (h w)")
            nc.sync.dma_start(
                x_flat[:].rearrange("p (n hw) -> p n hw", n=N), xsrc)

            # ---- W: [c, (f j)] layout keeps the DMA descriptors 36B ----
            w32 = sp.tile([128, FL * DCH], f32)
            wsrc = w_d.ap().rearrange("f c kh kw -> c f (kh kw)")
            nc.sync.dma_start(
                w32[:].rearrange("p (f j) -> p f j", f=FL), wsrc)
            wb = sp.tile([128, FL * DCH], bf16)
            nc.gpsimd.tensor_copy(wb[:], w32[:])
            nc.gpsimd.tensor_copy(w32[:], wb[:])
            w32n = sp.tile([128, FL * DCH], f32)
            nc.gpsimd.tensor_scalar(w32n[:], w32[:], -1.0, None, op0=Alu.mult)
            w32_3 = w32[:].rearrange("p (f j) -> p f j", f=FL)
            w32n_3 = w32n[:].rearrange("p (f j) -> p f j", f=FL)

            # ---- stationary / constant tiles ----
            ind = sp.tile([128, FL * FL], bf16)   # -2 at column f
            nc.gpsimd.memset(ind[:], 0.0)
            ind3 = ind[:].rearrange("p (f m) -> p f m", f=FL)
            for f in range(FL):
                nc.gpsimd.memset(ind3[:, f, f:f + 1], -2.0)
            # ---- padded x ----
            x_pad = sp.tile([128, N * HP * WP], f32)
            nc.gpsimd.memset(x_pad[:], 0.0)
            x_pad4 = x_pad[:].rearrange("p (n h w) -> p n h w", n=N, h=HP, w=WP)
            nc.scalar.copy(
                x_pad4[:, :, 1:1 + H, 1:1 + W_],
                x_flat[:].rearrange("p (n h w) -> p n h w", n=N, h=H, w=W_))

            # ---- the 9 shifted patch tiles (bf16), center first ----
            patches = [None] * DCH
            for k, j in enumerate(J_ORDER):
                dy, dx = divmod(j, KW)
                pj = sp.tile([128, L], bf16, tag=f"patch{j}")
                if j == 4:
                    nc.vector.tensor_copy(pj[:], x_flat[:])
                else:
                    pj4 = pj[:].rearrange(
                        "p (n h w) -> p n h w", n=N, h=H, w=W_)
                    src = x_pad4[:, :, dy:dy + H, dx:dx + W_]
                    if k % 2 == 1:
                        nc.vector.tensor_copy(pj4, src)
                    else:
                        nc.scalar.copy(pj4, src)
                patches[j] = pj

            # ---- S_X inputs: chain-add the 9 patches on GpSimd (slow
            #      but fully parallel to the DVE/ACT/PE pipeline) ----
            s01 = sp.tile([128, L], bf16)
            nc.gpsimd.tensor_tensor(s01[:], patches[0][:], patches[1][:],
                                    op=Alu.add)
            for j in [2, 3, 4, 5, 6, 7, 8]:
                nc.gpsimd.tensor_tensor(s01[:], s01[:], patches[j][:],
                                        op=Alu.add)

            psum = pp.tile([FL, L], f32)
            nchunks = L // NT

            # S_X: reduce sum-of-patches over partitions
            sxp = pp.tile([1, L], f32, tag="aux")
            for ncnk in range(nchunks):
                cs = slice(ncnk * NT, (ncnk + 1) * NT)
                nc.tensor.matmul(sxp[:, cs], ones_st[:, 0:1], s01[:, cs],
                                 start=True, stop=True)
            sxb = sp.tile([1, L], bf16)
            nc.scalar.copy(sxb[:], sxp[:])

            # S_W: reduce W over partitions, fold the 9 taps
            swp = pp.tile([1, FL * DCH], f32, tag="aux")
            nc.tensor.matmul(swp[:], ones_st[:, 0:1], wb[:],
                             start=True, stop=True)
            swf = sp.tile([1, FL * DCH], f32)
            nc.scalar.copy(swf[:], swp[:])
            swf3 = swf[:].rearrange("p (f j) -> p f j", f=FL)
            for k in range(1, DCH):
                nc.vector.tensor_tensor(
                    swf3[:, :, 0], swf3[:, :, 0], swf3[:, :, k], op=Alu.add)
            swb = sp.tile([1, FL], bf16)
            nc.vector.tensor_copy(swb[:], swf3[:, :, 0])

            # ---- main loop: relu tiles -> accumulating matmuls.
            #      MERGE_PAIRS tiles are pre-added on DVE so the pair
            #      costs one matmul pass instead of two. ----
            first = [True] * nchunks
            tidx = 0

            def relu_tile(f, j):
                nonlocal tidx
                dt_ = dp.tile([128, L], bf16, tag="diff")
                if tidx % ACT_MOD < ACT_K:
                    nc.scalar.activation(
                        dt_[:], patches[j][:], Act.Relu,
                        bias=w32n_3[:, f, j:j + 1], scale=1.0)
                else:
                    nc.vector.tensor_scalar(
                        dt_[:], patches[j][:], w32_3[:, f, j:j + 1], 0.0,
                        op0=Alu.subtract, op1=Alu.max)
                tidx += 1
                return dt_

            def feed_pe(dt_, lhsT):
                for ncnk in range(nchunks):
                    cs = slice(ncnk * NT, (ncnk + 1) * NT)
                    nc.tensor.matmul(
                        psum[:, cs], lhsT, dt_[:, cs],
                        start=first[ncnk], stop=False)
                    first[ncnk] = False

            for f in range(FL):
                lhsT = ind3[:, f, :]
                for j in SINGLE_J:
                    feed_pe(relu_tile(f, j), lhsT)
                for ja, jb in MERGE_PAIRS:
                    da = relu_tile(f, ja)
                    db = relu_tile(f, jb)
                    nc.vector.tensor_tensor(da[:], da[:], db[:], op=Alu.add)
                    feed_pe(da, lhsT)

            # ---- broadcast corrections into psum with K=1 matmuls,
            #      then stream each finished chunk straight out ----
            osb = sp.tile([FL, L], f32)
            odst = out_d.ap().rearrange("n f h w -> f n (h w)")
            osb3 = osb[:].rearrange("f (n hw) -> f n hw", n=N)
            for ncnk in range(nchunks):
                cs = slice(ncnk * NT, (ncnk + 1) * NT)
                nc.tensor.matmul(                      # += S_X[l] every row
                    psum[:, cs], ones_st[0:1, :], sxb[:, cs],
                    start=False, stop=False)
                nc.tensor.matmul(                      # += -S_W[f] every col
                    psum[:, cs], swb[:], neg_ones[0:1, :],
                    start=False, stop=True)
                nc.scalar.copy(osb[:, cs], psum[:, cs])
            nc.sync.dma_start(odst, osb3)

    _dedup_ldweights(nc)
    nc.compile()
    return nc


def kernel(x, W):
    x = np.ascontiguousarray(np.asarray(x, dtype=np.float32))
    W = np.ascontiguousarray(np.asarray(W, dtype=np.float32))
    assert x.shape == (N, C, H, W_) and W.shape == (F, C, KH, KW)

    if "nc" not in _CACHE:
        _CACHE["nc"] = _build_nc()
    nc = _CACHE["nc"]

    from concourse.bass_utils import run_bass_kernel_spmd

    in_maps = [
        {"x": x, "w": np.ascontiguousarray(W[FL * i:FL * (i + 1)])}
        for i in range(NCORES)
    ]
    trace = bool(_CACHE.get("trace", False))
    res = run_bass_kernel_spmd(nc, in_maps, core_ids=list(range(NCORES)),
                               trace=trace)
    _CACHE["exec_time_ns"] = res.exec_time_ns
    out = np.concatenate([r["out"] for r in res.results], axis=1)
    return out.astype(np.float32)


# revision 25
# speedup vs baseline: 1.1743x; 1.1540x over previous
"""Adder2D (L1-distance "convolution") Trainium2 Bass kernel, 8 NeuronCores.

out[n, f, ho, wo] = -sum_d |W[f, d] - X_col[d, (n, ho, wo)]|
with d = (c, dy, dx), C=128, 3x3 kernel, stride 1, pad 1.

Sharding: output-channel tensor parallel. Core i computes filters
[16*i, 16*(i+1)); every core sees the full x. No collectives; the host
concatenates the 8 per-core outputs along the filter axis.

Per-core algorithm (relu identity, exact):
  |x-w| = 2*relu(x-w) - (x-w)
  out[f, l] = -2*sum_d relu(x - w[f,d]) + S_X[l] - S_W[f]
  S_X[l] = sum_d x[d, l],  S_W[f] = sum_d w[f, d]

  - x (f32) is DMA'd contiguously, then zero-padded into
    [128c, 8n*18*18] on-chip.
  - 9 shifted copies materialize im2col patches as bf16 [128c, 2048l];
    the "unfold" is just an access pattern (center patch first: it has
    no dependency on the padding, so the PE pipeline starts early).
  - relu tiles: DVE tensor_scalar(op0=subtract, op1=max, 0.0) with a
    per-partition f32 W scalar (4x bf16 path), ~75% of tiles; ACT
    activation(Relu, bias=-w) for the rest.
  - TensorEngine reduces over partitions with accumulating matmuls into
    one [16, 2048] PSUM tile; stationary = [128,16] column of -2 at
    column f. A custom pass drops LDWEIGHTS whose stationary is
    unchanged (else walrus reloads it for every matmul: +38% PE time).
  - S_X: tree-add of the 9 patches (DVE/GpSimd) + a ones-column matmul;
    S_W: ones-column matmul over W + a 9-fold strided add. Both are
    broadcast back into the PSUM accumulator with K=1 matmuls, so the
    corrections cost ~13 matmuls instead of 72.
"""

import numpy as np

N, C, H, W_ = 8, 128, 16, 16
F, KH, KW = 128, 3, 3
NCORES = 8
FL = F // NCORES          # 16 filters per core
HP, WP = H + 2, W_ + 2    # padded 18x18
L = N * H * W_            # 2048 output columns
DCH = KH * KW             # 9 shift chunks of 128 channels
NT = 512                  # matmul moving free dim (one PSUM bank)
ACT_MOD, ACT_K = 3, 1     # 1 of every 3 relu tiles runs on the Scalar engine
WARMUP_MM = 16            # PE warmup matmuls during the DMA/setup phase
MERGE_PAIRS = [(0, 1), (2, 3)]          # relu tiles pre-added on DVE
SINGLE_J = [4, 5, 6, 7, 8]              # tiles fed to PE directly
J_ORDER = [4, 0, 1, 2, 3, 5, 6, 7, 8]   # center shift first (patch creation)

_CACHE = {}


def _dedup_ldweights(nc):
    """Drop InstLdweights whose stationary operand is identical to the
    previous weight load on the PE stream (the array keeps its weights
    between matmuls; per-matmul reloads of an unchanged stationary are
    pure overhead). Runs after Tile scheduling, before bacc.compile,
    when the ldweights carry no semaphore sync."""
    from concourse import mybir
    removed = 0
    for fn in nc.m.functions:
        for blk in fn.blocks:
            last_key = None
            keep = []
            for inst in blk.instructions:
                if isinstance(inst, mybir.InstLdweights):
                    si = inst.sync_info
                    clean = si is None or (not si.on_wait and not si.on_update)
                    key = "|".join(str(s) for s in (
                        inst.ins[0], inst.perf_mode, inst.is_transpose,
                        inst.tile_position, inst.tile_size))
                    if clean and key == last_key:
                        removed += 1
                        continue
                    last_key = key
                keep.append(inst)
            blk.instructions[:] = keep
    return removed


def _build_nc():
    from concourse import bacc, mybir
    import concourse.tile as tile

    f32 = mybir.dt.float32
    bf16 = mybir.dt.bfloat16
    Alu = mybir.AluOpType
    Act = mybir.ActivationFunctionType

    nc = bacc.Bacc("TRN2", target_bir_lowering=False, debug=False,
                   num_devices=NCORES)
    x_d = nc.dram_tensor("x", [N, C, H, W_], f32, kind="ExternalInput")
    w_d = nc.dram_tensor("w", [FL, C, KH, KW], f32, kind="ExternalInput")
    out_d = nc.dram_tensor("out", [N, FL, H, W_], f32, kind="ExternalOutput")

    with tile.TileContext(nc) as tc:
        with tc.tile_pool(name="setup", bufs=1) as sp, \
             tc.tile_pool(name="diff", bufs=8) as dp, \
             tc.tile_pool(name="psum", bufs=1, space="PSUM") as pp:

            # ---- PE warmup first: constants on DVE, then matmuls that
            #      keep HAM at 2.4 GHz while the DMAs/setup run ----
            ones_st = sp.tile([128, FL], bf16)
            nc.vector.memset(ones_st[:], 1.0)
            neg_ones = sp.tile([128, NT], bf16)
            nc.vector.memset(neg_ones[:], -1.0)
            warm = pp.tile([FL, NT], f32, tag="aux")
            for i in range(WARMUP_MM):
                nc.tensor.matmul(warm[:], ones_st[:], neg_ones[:],
                                 start=(i == 0), stop=(i == WARMUP_MM - 1))

            # ---- x: contiguous DMA ----
            x_flat = sp.tile([128, L], f32)
            xsrc = x_d.ap().rearrange("n c h w -> c n (h w)")
            nc.sync.dma_start(
                x_flat[:].rearrange("p (n hw) -> p n hw", n=N), xsrc)

            # ---- W: [c, (f j)] layout keeps the DMA descriptors 36B ----
            w32 = sp.tile([128, FL * DCH], f32)
            wsrc = w_d.ap().rearrange("f c kh kw -> c f (kh kw)")
            nc.sync.dma_start(
                w32[:].rearrange("p (f j) -> p f j", f=FL), wsrc)
            wb = sp.tile([128, FL * DCH], bf16)
            nc.gpsimd.tensor_copy(wb[:], w32[:])
            nc.gpsimd.tensor_copy(w32[:], wb[:])
            w32n = sp.tile([128, FL * DCH], f32)
            nc.gpsimd.tensor_scalar(w32n[:], w32[:], -1.0, None, op0=Alu.mult)
            w32_3 = w32[:].rearrange("p (f j) -> p f j", f=FL)
            w32n_3 = w32n[:].rearrange("p (f j) -> p f j", f=FL)

            # ---- stationary / constant tiles ----
            ind = sp.tile([128, FL * FL], bf16)   # -2 at column f
            nc.gpsimd.memset(ind[:], 0.0)
            ind3 = ind[:].rearrange("p (f m) -> p f m", f=FL)
            for f in range(FL):
                nc.gpsimd.memset(ind3[:, f, f:f + 1], -2.0)
            # ---- padded x ----
            x_pad = sp.tile([128, N * HP * WP], f32)
            nc.gpsimd.memset(x_pad[:], 0.0)
            x_pad4 = x_pad[:].rearrange("p (n h w) -> p n h w", n=N, h=HP, w=WP)
            nc.scalar.copy(
                x_pad4[:, :, 1:1 + H, 1:1 + W_],
                x_flat[:].rearrange("p (n h w) -> p n h w", n=N, h=H, w=W_))

            # ---- the 9 shifted patch tiles (bf16), center first ----
            patches = [None] * DCH
            for k, j in enumerate(J_ORDER):
                dy, dx = divmod(j, KW)
                pj = sp.tile([128, L], bf16, tag=f"patch{j}")
                if j == 4:
                    nc.vector.tensor_copy(pj[:], x_flat[:])
                else:
                    pj4 = pj[:].rearrange(
                        "p (n h w) -> p n h w", n=N, h=H, w=W_)
                    src = x_pad4[:, :, dy:dy + H, dx:dx + W_]
                    if k % 2 == 1:
                        nc.vector.tensor_copy(pj4, src)
                    else:
                        nc.scalar.copy(pj4, src)
                patches[j] = pj

            psum = pp.tile([FL, L], f32)
            nchunks = L // NT

            # ---- main loop: relu tiles -> accumulating matmuls.
            #      MERGE_PAIRS tiles are pre-added on DVE so the pair
            #      costs one matmul pass instead of two. ----
            first = [True] * nchunks
            tidx = 0

            def relu_tile(f, j):
                nonlocal tidx
                dt_ = dp.tile([128, L], bf16, tag="diff")
                if tidx % ACT_MOD < ACT_K:
                    nc.scalar.activation(
                        dt_[:], patches[j][:], Act.Relu,
                        bias=w32n_3[:, f, j:j + 1], scale=1.0)
                else:
                    nc.vector.tensor_scalar(
                        dt_[:], patches[j][:], w32_3[:, f, j:j + 1], 0.0,
                        op0=Alu.subtract, op1=Alu.max)
                tidx += 1
                return dt_

            def feed_pe(dt_, lhsT):
                for ncnk in range(nchunks):
                    cs = slice(ncnk * NT, (ncnk + 1) * NT)
                    nc.tensor.matmul(
                        psum[:, cs], lhsT, dt_[:, cs],
                        start=first[ncnk], stop=False)
                    first[ncnk] = False

            for f in range(FL):
                lhsT = ind3[:, f, :]
                for j in SINGLE_J:
                    feed_pe(relu_tile(f, j), lhsT)
                for ja, jb in MERGE_PAIRS:
                    da = relu_tile(f, ja)
                    db = relu_tile(f, jb)
                    nc.vector.tensor_tensor(da[:], da[:], db[:], op=Alu.add)
                    feed_pe(da, lhsT)

            # ---- corrections (all emitted after the main loop: engine
            #      queues are FIFO, so anything waiting here cannot
            #      block the main pipeline) ----
            # S_X: += sum_d(chunk j) x[d, l] for every row, one
            # stationary (ones) for all 36 matmuls
            for j in range(DCH):
                for ncnk in range(nchunks):
                    cs = slice(ncnk * NT, (ncnk + 1) * NT)
                    nc.tensor.matmul(
                        psum[:, cs], ones_st[:], patches[j][:, cs],
                        start=False, stop=False)

            # S_W: reduce W over partitions, fold the 9 taps
            swp = pp.tile([1, FL * DCH], f32, tag="aux")
            nc.tensor.matmul(swp[:], ones_st[:, 0:1], wb[:],
                             start=True, stop=True)
            swf = sp.tile([1, FL * DCH], f32)
            nc.scalar.copy(swf[:], swp[:])
            swf3 = swf[:].rearrange("p (f j) -> p f j", f=FL)
            for k in range(1, DCH):
                nc.vector.tensor_tensor(
                    swf3[:, :, 0], swf3[:, :, 0], swf3[:, :, k], op=Alu.add)
            swb = sp.tile([1, FL], bf16)
            nc.vector.tensor_copy(swb[:], swf3[:, :, 0])

            # broadcast -S_W into psum with K=1 matmuls, then stream
            # each finished chunk straight out
            osb = sp.tile([FL, L], f32)
            odst = out_d.ap().rearrange("n f h w -> f n (h w)")
            osb3 = osb[:].rearrange("f (n hw) -> f n hw", n=N)
            for ncnk in range(nchunks):
                cs = slice(ncnk * NT, (ncnk + 1) * NT)
                nc.tensor.matmul(                      # += -S_W[f] every col
                    psum[:, cs], swb[:], neg_ones[0:1, :],
                    start=False, stop=True)
                nc.scalar.copy(osb[:, cs], psum[:, cs])
            nc.sync.dma_start(odst, osb3)

    _dedup_ldweights(nc)
    nc.compile()
    return nc


def kernel(x, W):
    x = np.ascontiguousarray(np.asarray(x, dtype=np.float32))
    W = np.ascontiguousarray(np.asarray(W, dtype=np.float32))
    assert x.shape == (N, C, H, W_) and W.shape == (F, C, KH, KW)

    if "nc" not in _CACHE:
        _CACHE["nc"] = _build_nc()
    nc = _CACHE["nc"]

    from concourse.bass_utils import run_bass_kernel_spmd

    in_maps = [
        {"x": x, "w": np.ascontiguousarray(W[FL * i:FL * (i + 1)])}
        for i in range(NCORES)
    ]
    trace = bool(_CACHE.get("trace", False))
    res = run_bass_kernel_spmd(nc, in_maps, core_ids=list(range(NCORES)),
                               trace=trace)
    _CACHE["exec_time_ns"] = res.exec_time_ns
    out = np.concatenate([r["out"] for r in res.results], axis=1)
    return out.astype(np.float32)


# revision 29
# speedup vs baseline: 1.1783x; 1.0034x over previous
"""Adder2D (L1-distance "convolution") Trainium2 Bass kernel, 8 NeuronCores.

out[n, f, ho, wo] = -sum_d |W[f, d] - X_col[d, (n, ho, wo)]|
with d = (c, dy, dx), C=128, 3x3 kernel, stride 1, pad 1.

Sharding: output-channel tensor parallel. Core i computes filters
[16*i, 16*(i+1)); every core sees the full x. No collectives; the host
concatenates the 8 per-core outputs along the filter axis.

Per-core algorithm (relu identity, exact):
  |x-w| = 2*relu(x-w) - (x-w)
  out[f, l] = -2*sum_d relu(x - w[f,d]) + S_X[l] - S_W[f]
  S_X[l] = sum_d x[d, l],  S_W[f] = sum_d w[f, d]

  - x (f32) is DMA'd contiguously, then zero-padded into
    [128c, 8n*18*18] on-chip.
  - 9 shifted copies materialize im2col patches as bf16 [128c, 2048l];
    the "unfold" is just an access pattern (center patch first: it has
    no dependency on the padding, so the PE pipeline starts early).
  - relu tiles: DVE tensor_scalar(op0=subtract, op1=max, 0.0) with a
    per-partition f32 W scalar (4x bf16 path), ~75% of tiles; ACT
    activation(Relu, bias=-w) for the rest.
  - TensorEngine reduces over partitions with accumulating matmuls into
    one [16, 2048] PSUM tile; stationary = [128,16] column of -2 at
    column f. A custom pass drops LDWEIGHTS whose stationary is
    unchanged (else walrus reloads it for every matmul: +38% PE time).
  - S_X: tree-add of the 9 patches (DVE/GpSimd) + a ones-column matmul;
    S_W: ones-column matmul over W + a 9-fold strided add. Both are
    broadcast back into the PSUM accumulator with K=1 matmuls, so the
    corrections cost ~13 matmuls instead of 72.
"""

import numpy as np

N, C, H, W_ = 8, 128, 16, 16
F, KH, KW = 128, 3, 3
NCORES = 8
FL = F // NCORES          # 16 filters per core
HP, WP = H + 2, W_ + 2    # padded 18x18
L = N * H * W_            # 2048 output columns
DCH = KH * KW             # 9 shift chunks of 128 channels
NT = 512                  # matmul moving free dim (one PSUM bank)
ACT_MOD, ACT_K = 3, 1     # 1 of every 3 relu tiles runs on the Scalar engine
WARMUP_MM = 16            # PE warmup matmuls during the DMA/setup phase
MERGE_PAIRS = [(0, 1), (2, 3)]          # relu tiles pre-added on DVE
SINGLE_J = [4, 5, 6, 7, 8]              # tiles fed to PE directly
J_ORDER = [4, 0, 1, 2, 3, 5, 6, 7, 8]   # center shift first (patch creation)

_CACHE = {}


def _dedup_ldweights(nc):
    """Drop InstLdweights whose stationary operand is identical to the
    previous weight load on the PE stream (the array keeps its weights
    between matmuls; per-matmul reloads of an unchanged stationary are
    pure overhead). Runs after Tile scheduling, before bacc.compile,
    when the ldweights carry no semaphore sync."""
    from concourse import mybir
    removed = 0
    for fn in nc.m.functions:
        for blk in fn.blocks:
            last_key = None
            keep = []
            for inst in blk.instructions:
                if isinstance(inst, mybir.InstLdweights):
                    si = inst.sync_info
                    clean = si is None or (not si.on_wait and not si.on_update)
                    key = "|".join(str(s) for s in (
                        inst.ins[0], inst.perf_mode, inst.is_transpose,
                        inst.tile_position, inst.tile_size))
                    if clean and key == last_key:
                        removed += 1
                        continue
                    last_key = key
                keep.append(inst)
            blk.instructions[:] = keep
    return removed


def _build_nc():
    from concourse import bacc, mybir
    import concourse.tile as tile

    f32 = mybir.dt.float32
    bf16 = mybir.dt.bfloat16
    Alu = mybir.AluOpType
    Act = mybir.ActivationFunctionType

    nc = bacc.Bacc("TRN2", target_bir_lowering=False, debug=False,
                   num_devices=NCORES)
    x_d = nc.dram_tensor("x", [N, C, H, W_], f32, kind="ExternalInput")
    w_d = nc.dram_tensor("w", [FL, C, KH, KW], f32, kind="ExternalInput")
    out_d = nc.dram_tensor("out", [N, FL, H, W_], f32, kind="ExternalOutput")

    with tile.TileContext(nc) as tc:
        with tc.tile_pool(name="setup", bufs=1) as sp, \
             tc.tile_pool(name="diff", bufs=8) as dp, \
             tc.tile_pool(name="psum", bufs=1, space="PSUM") as pp:

            # ---- PE warmup first: constants on DVE, then matmuls that
            #      keep HAM at 2.4 GHz while the DMAs/setup run ----
            ones_st = sp.tile([128, FL], bf16)
            nc.vector.memset(ones_st[:], 1.0)
            neg_ones = sp.tile([128, NT], bf16)
            nc.vector.memset(neg_ones[:], -1.0)
            # preload the ACT spline tables before the first real Relu
            actwarm = sp.tile([1, 16], f32)
            nc.scalar.activation(actwarm[:], ones_st[0:1, 0:16], Act.Relu)
            warm = pp.tile([FL, NT], f32, tag="aux")
            for i in range(WARMUP_MM):
                nc.tensor.matmul(warm[:], ones_st[:], neg_ones[:],
                                 start=(i == 0), stop=(i == WARMUP_MM - 1))

            # ---- W first (tiny), then x: contiguous DMAs ----
            w_raw = sp.tile([FL, C * DCH], f32)
            nc.sync.dma_start(w_raw[:], w_d.ap().rearrange(
                "f c kh kw -> f (c kh kw)"))
            x_flat = sp.tile([128, L], f32)
            xsrc = x_d.ap().rearrange("n c h w -> c n (h w)")
            nc.sync.dma_start(
                x_flat[:].rearrange("p (n hw) -> p n hw", n=N), xsrc)

            # ---- W transposed on the (idle) PE: 9 shifts of [16, 128]
            #      -> [128c, (j f)] in PSUM, then cast chain on DVE ----
            from concourse.masks import make_identity
            ident = sp.tile([FL, FL], f32)
            make_identity(nc, ident[:])
            wtp = pp.tile([128, DCH * FL], f32, tag="wt")
            w_raw3 = w_raw[:].rearrange("p (c j) -> p c j", j=DCH)
            for j in range(DCH):
                nc.tensor.matmul(
                    wtp[:, FL * j:FL * (j + 1)], w_raw3[:, :, j], ident[:],
                    is_transpose=True, start=True, stop=True)
            w32 = sp.tile([128, DCH * FL], f32)
            nc.vector.tensor_copy(w32[:], wtp[:])
            wb = sp.tile([128, DCH * FL], bf16)
            nc.vector.tensor_copy(wb[:], w32[:])
            nc.vector.tensor_copy(w32[:], wb[:])
            w32n = sp.tile([128, DCH * FL], f32)
            nc.vector.tensor_scalar(w32n[:], w32[:], -1.0, None, op0=Alu.mult)
            w32_3 = w32[:].rearrange("p (j f) -> p j f", j=DCH)
            w32n_3 = w32n[:].rearrange("p (j f) -> p j f", j=DCH)

            # ---- stationary / constant tiles ----
            ind = sp.tile([128, FL * FL], bf16)   # -2 at column f
            nc.gpsimd.memset(ind[:], 0.0)
            ind3 = ind[:].rearrange("p (f m) -> p f m", f=FL)
            for f in range(FL):
                nc.gpsimd.memset(ind3[:, f, f:f + 1], -2.0)
            # ---- padded x ----
            x_pad = sp.tile([128, N * HP * WP], f32)
            nc.gpsimd.memset(x_pad[:], 0.0)
            x_pad4 = x_pad[:].rearrange("p (n h w) -> p n h w", n=N, h=HP, w=WP)
            nc.scalar.copy(
                x_pad4[:, :, 1:1 + H, 1:1 + W_],
                x_flat[:].rearrange("p (n h w) -> p n h w", n=N, h=H, w=W_))

            # ---- the 9 shifted patch tiles (bf16), center first ----
            patches = [None] * DCH
            for k, j in enumerate(J_ORDER):
                dy, dx = divmod(j, KW)
                pj = sp.tile([128, L], bf16, tag=f"patch{j}")
                if j == 4:
                    nc.vector.tensor_copy(pj[:], x_flat[:])
                else:
                    pj4 = pj[:].rearrange(
                        "p (n h w) -> p n h w", n=N, h=H, w=W_)
                    src = x_pad4[:, :, dy:dy + H, dx:dx + W_]
                    if k % 2 == 1:
                        nc.vector.tensor_copy(pj4, src)
                    else:
                        nc.scalar.copy(pj4, src)
                patches[j] = pj

            psum = pp.tile([FL, L], f32)
            nchunks = L // NT

            # ---- main loop: relu tiles -> accumulating matmuls.
            #      MERGE_PAIRS tiles are pre-added on DVE so the pair
            #      costs one matmul pass instead of two. ----
            first = [True] * nchunks
            tidx = 0

            def relu_tile(f, j):
                nonlocal tidx
                dt_ = dp.tile([128, L], bf16, tag="diff")
                if tidx % ACT_MOD < ACT_K:
                    nc.scalar.activation(
                        dt_[:], patches[j][:], Act.Relu,
                        bias=w32n_3[:, j, f:f + 1], scale=1.0)
                else:
                    nc.vector.tensor_scalar(
                        dt_[:], patches[j][:], w32_3[:, j, f:f + 1], 0.0,
                        op0=Alu.subtract, op1=Alu.max)
                tidx += 1
                return dt_

            def feed_pe(dt_, lhsT):
                for ncnk in range(nchunks):
                    cs = slice(ncnk * NT, (ncnk + 1) * NT)
                    nc.tensor.matmul(
                        psum[:, cs], lhsT, dt_[:, cs],
                        start=first[ncnk], stop=False)
                    first[ncnk] = False

            for f in range(FL):
                lhsT = ind3[:, f, :]
                for j in SINGLE_J:
                    feed_pe(relu_tile(f, j), lhsT)
                for ja, jb in MERGE_PAIRS:
                    da = relu_tile(f, ja)
                    db = relu_tile(f, jb)
                    nc.vector.tensor_tensor(da[:], da[:], db[:], op=Alu.add)
                    feed_pe(da, lhsT)

            # ---- corrections (all emitted after the main loop: engine
            #      queues are FIFO, so anything waiting here cannot
            #      block the main pipeline) ----
            # S_X: += sum_d(chunk j) x[d, l] for every row, one
            # stationary (ones) for all 36 matmuls
            for j in range(DCH):
                for ncnk in range(nchunks):
                    cs = slice(ncnk * NT, (ncnk + 1) * NT)
                    nc.tensor.matmul(
                        psum[:, cs], ones_st[:], patches[j][:, cs],
                        start=False, stop=False)

            # S_W: reduce W over partitions, fold the 9 taps
            swp = pp.tile([1, FL * DCH], f32, tag="aux")
            nc.tensor.matmul(swp[:], ones_st[:, 0:1], wb[:],
                             start=True, stop=True)
            swf = sp.tile([1, FL * DCH], f32)
            nc.scalar.copy(swf[:], swp[:])
            swf3 = swf[:].rearrange("p (j f) -> p j f", j=DCH)
            for k in range(1, DCH):
                nc.vector.tensor_tensor(
                    swf3[:, 0, :], swf3[:, 0, :], swf3[:, k, :], op=Alu.add)
            swb = sp.tile([1, FL], bf16)
            nc.vector.tensor_copy(swb[:], swf3[:, 0, :])

            # broadcast -S_W into psum with K=1 matmuls, then stream
            # each finished chunk straight out
            osb = sp.tile([FL, L], f32)
            odst = out_d.ap().rearrange("n f h w -> f n (h w)")
            osb3 = osb[:].rearrange("f (n hw) -> f n hw", n=N)
            for ncnk in range(nchunks):
                cs = slice(ncnk * NT, (ncnk + 1) * NT)
                nc.tensor.matmul(                      # += -S_W[f] every col
                    psum[:, cs], swb[:], neg_ones[0:1, :],
                    start=False, stop=True)
                nc.scalar.copy(osb[:, cs], psum[:, cs])
            nc.sync.dma_start(odst, osb3)

    _dedup_ldweights(nc)
    nc.compile()
    return nc


def kernel(x, W):
    x = np.ascontiguousarray(np.asarray(x, dtype=np.float32))
    W = np.ascontiguousarray(np.asarray(W, dtype=np.float32))
    assert x.shape == (N, C, H, W_) and W.shape == (F, C, KH, KW)

    if "nc" not in _CACHE:
        _CACHE["nc"] = _build_nc()
    nc = _CACHE["nc"]

    from concourse.bass_utils import run_bass_kernel_spmd

    in_maps = [
        {"x": x, "w": np.ascontiguousarray(W[FL * i:FL * (i + 1)])}
        for i in range(NCORES)
    ]
    trace = bool(_CACHE.get("trace", False))
    res = run_bass_kernel_spmd(nc, in_maps, core_ids=list(range(NCORES)),
                               trace=trace)
    _CACHE["exec_time_ns"] = res.exec_time_ns
    out = np.concatenate([r["out"] for r in res.results], axis=1)
    return out.astype(np.float32)
